# revision 3
# baseline (speedup 1.0000x reference)
"""Trainium2 Bass kernel for nn_HNetEnergyViaBoolWeights.

Reference computation:
    a[n,e] = act[n, idx[e,0]],  b[n,e] = act[n, idx[e,1]]        (act is 0/1)
    code[n,e] = TEMP_TO_CODE[2a+b];  bact = one_hot(code)        (type-major)
    energies[n,c] = sum_k binarized_learned[c,k] * bact[n,k]
    out = energies - min(energies)

Key identity: with the 5 learned one-hot planes P0..P4 (over edge types
[0,2,3,5,9]) and temp = 2a+b in {0..3}, the per-edge match P_{temp+1}[c,e]
expands over the multilinear basis {1, a, b, ab}:

    match = P1 + (P3-P1)*a + (P2-P1)*b + (P4-P3-P2+P1)*ab

All three data-dependent coefficient planes have values in {-1,0,1}
(exactly representable in fp8); the constant plane contributes the
data-independent bias K[c] = sum_e P1[c,e] (folded in on the host).

Sharding: edges split across the 8 NeuronCores (6250 each); every core
computes a partial energy (64 cmp x 512 pts) over its edge slice.  The host
sums the 8 partials, adds K, and applies the global min subtraction.

Per-core device program (pipelined):
  - edges processed in 13 groups (12x4 + 1 chunks of 128 edges); per group
    two dma_gathers (a/b endpoint rows of the fp8 activation table, landing
    edges on K-partitions) spread over all 4 SWDGE queues, one DVE
    bitwise-AND forming the a*b plane, then fp8 DoubleRow matmuls (two
    128-edge chunks per instruction) accumulating into one PSUM bank.
  - 4 SWDGE queues + 6-deep tile buffering keep the DMA engines saturated
    (the gather is descriptor-bound: ~12.5k 512B descriptors/core).
"""

import numpy as np
import ml_dtypes

N_PTS = 512
N_NODES = 10000
N_EDGES = 50000
N_CMP = 64
N_CORES = 8
EDGES_PER_CORE = N_EDGES // N_CORES          # 6250
GROUPS = [4] * 12 + [1]                      # chunks (of 128 edges) per group
J_CHUNKS = sum(GROUPS)                       # 49
EDGES_PAD = J_CHUNKS * 128                   # 6272
CJ_MAX = max(GROUPS)
N_PLANES = 3                                 # a, b, ab coefficient planes
N_BUFS = 6
F8 = ml_dtypes.float8_e4m3

_compiled = None


def _build_bass(repeats=1, loop_iters=0):
    """Build + compile the per-core Bass program (shared by all 8 cores).

    repeats>1 unrolls the body; loop_iters>0 wraps it in a device-side For_i
    loop - both used only by the test harness to measure per-iteration device
    time by wall-clock differencing.
    """
    import concourse.mybir as mybir
    import concourse.tile as tile
    from concourse import bacc
    from concourse.library_config import mlp
    from contextlib import nullcontext

    dt = mybir.dt
    nc = bacc.Bacc("TRN2", target_bir_lowering=False, debug=False,
                   num_devices=N_CORES, num_swdge_queues=4)

    acts = nc.dram_tensor("acts", [N_NODES, N_PTS], dt.float8e4,
                          kind="ExternalInput")
    # columns [0:392) = idx_a wrapped, [392:784) = idx_b wrapped
    idx_ab = nc.dram_tensor("idx_ab", [128, 2 * EDGES_PAD // 16], dt.int16,
                            kind="ExternalInput")
    # wp[p, 3*j0_g + t*cj_g + jj, c] = coeff_t[c, (j0_g+jj)*128 + p]
    wplanes = nc.dram_tensor("wplanes", [128, N_PLANES * J_CHUNKS, N_CMP],
                             dt.float8e4, kind="ExternalInput")
    partial = nc.dram_tensor("partial", [N_CMP, N_PTS], dt.float32,
                             kind="ExternalOutput")

    DR = mybir.MatmulPerfMode.DoubleRow
    IDXW = EDGES_PAD // 16                   # 392 wrapped idx columns

    with tile.TileContext(nc) as tc:
        with tc.tile_pool(name="sbuf", bufs=1) as pool, \
             tc.tile_pool(name="gbuf", bufs=N_BUFS) as gpool, \
             tc.tile_pool(name="psum", bufs=1, space="PSUM") as psum_pool:
            nc.gpsimd.load_library(mlp)
            loop_cm = tc.For_i(0, loop_iters, 1) if loop_iters else nullcontext()
            with loop_cm:
                for rep in range(repeats):
                    idx_sb = pool.tile([128, 2 * IDXW], dt.int16,
                                       tag="idx_sb", bufs=2)
                    wp_sb = pool.tile([128, N_PLANES * J_CHUNKS, N_CMP],
                                      dt.float8e4, tag="wp_sb", bufs=2)
                    nc.sync.dma_start(idx_sb[:], idx_ab[:])
                    nc.sync.dma_start(wp_sb[:], wplanes[:])

                    acc = psum_pool.tile([N_CMP, N_PTS], dt.float32,
                                         tag="acc")

                    n_mm = N_PLANES * J_CHUNKS
                    k_mm = 0   # running chunk count for start/stop flags
                    j0 = 0     # first chunk of this group
                    for g, cj in enumerate(GROUPS):
                        ga = gpool.tile([128, cj, N_PTS], dt.float8e4,
                                        tag="ga",
                                        padded_shape=[128, CJ_MAX, N_PTS])
                        gb = gpool.tile([128, cj, N_PTS], dt.float8e4,
                                        tag="gb",
                                        padded_shape=[128, CJ_MAX, N_PTS])
                        gab = gpool.tile([128, cj, N_PTS], dt.float8e4,
                                         tag="gab",
                                         padded_shape=[128, CJ_MAX, N_PTS])
                        sp = cj * 128 < 1024
                        nc.gpsimd.dma_gather(
                            ga[:], acts[:],
                            idx_sb[:, j0 * 8:(j0 + cj) * 8],
                            cj * 128, cj * 128, N_PTS,
                            single_packet=sp, queue_num=(2 * g) % 4)
                        nc.gpsimd.dma_gather(
                            gb[:], acts[:],
                            idx_sb[:, IDXW + j0 * 8:IDXW + (j0 + cj) * 8],
                            cj * 128, cj * 128, N_PTS,
                            single_packet=sp, queue_num=(2 * g + 1) % 4)
                        # ab plane: fp8 0/1 values AND bit-exactly; uint16
                        # pairs run the DVE in its 2x 16-bit mode.
                        nc.vector.tensor_tensor(
                            gab[:].bitcast(dt.uint16),
                            ga[:].bitcast(dt.uint16),
                            gb[:].bitcast(dt.uint16),
                            op=mybir.AluOpType.bitwise_and)

                        for t, src in enumerate((ga, gb, gab)):
                            base = (j0 * N_PLANES) + t * cj
                            for p in range(cj // 2):
                                nc.tensor.matmul(
                                    acc[:],
                                    wp_sb[:, base + 2 * p:base + 2 * p + 2, :],
                                    src[:, 2 * p:2 * p + 2, :],
                                    start=(k_mm == 0),
                                    stop=(k_mm + 2 == n_mm),
                                    perf_mode=DR)
                                k_mm += 2
                            if cj % 2:
                                nc.tensor.matmul(
                                    acc[:],
                                    wp_sb[:, base + cj - 1, :],
                                    src[:, cj - 1, :],
                                    start=(k_mm == 0),
                                    stop=(k_mm + 1 == n_mm))
                                k_mm += 1
                        j0 += cj

                    out_sb = pool.tile([N_CMP, N_PTS], dt.float32,
                                       tag="out_sb")
                    nc.vector.tensor_copy(out_sb[:], acc[:])
                    nc.sync.dma_start(partial[:], out_sb[:])

    nc.compile()
    return nc


def _get_compiled():
    global _compiled
    if _compiled is None:
        _compiled = _build_bass()
    return _compiled


def _wrap_idx(idx_slice):
    """int edge-endpoint array -> (128, EDGES_PAD//16) int16 wrapped layout
    (index i at partition i%16, column i//16; 16-row block replicated 8x)."""
    pad = np.zeros(EDGES_PAD, np.int16)
    pad[:idx_slice.shape[0]] = idx_slice.astype(np.int16)
    arr16 = pad.reshape(EDGES_PAD // 16, 16).T.copy()
    return np.tile(arr16, (8, 1))


def prepare_in_maps(node_activations, binarized_learned, edge_endnode_idx):
    """Host-side sharding/relayout. Returns (in_maps, K_bias)."""
    act = np.asarray(node_activations)
    W = np.asarray(binarized_learned)
    idx = np.asarray(edge_endnode_idx)

    # fp8 node-activation table, transposed: row v = act[:, v] over 512 pts
    acts_t = np.ascontiguousarray(act.T).astype(np.float32).astype(F8)

    # weight planes over EDGE_TYPES=[0,2,3,5,9]; multilinear coefficients
    P = W.reshape(N_CMP, 5, N_EDGES)
    P1, P2, P3, P4 = P[:, 1], P[:, 2], P[:, 3], P[:, 4]
    K_bias = P1.sum(axis=1, dtype=np.float64).astype(np.float32)  # (64,)
    coeff_a = P3 - P1
    coeff_b = P2 - P1
    coeff_ab = P4 - P3 - P2 + P1

    in_maps = []
    for s in range(N_CORES):
        sl = slice(s * EDGES_PER_CORE, (s + 1) * EDGES_PER_CORE)
        cpad = np.zeros((N_PLANES, N_CMP, EDGES_PAD), np.float32)
        for t, cf in enumerate((coeff_a, coeff_b, coeff_ab)):
            cpad[t, :, :EDGES_PER_CORE] = cf[:, sl]
        # slot layout (middle dim of wp_sb): group-major, then plane, then
        # chunk-within-group: slot(g, t, jj) = 3*j0_g + t*cj_g + jj
        ck = cpad.reshape(N_PLANES, N_CMP, J_CHUNKS, 128)
        wp = np.zeros((128, N_PLANES * J_CHUNKS, N_CMP), np.float32)
        j0 = 0
        for cj in GROUPS:
            for t in range(N_PLANES):
                # (c, jj, p) -> (p, jj, c)
                wp[:, 3 * j0 + t * cj:3 * j0 + (t + 1) * cj, :] = \
                    ck[t, :, j0:j0 + cj, :].transpose(2, 1, 0)
            j0 += cj
        in_maps.append({
            "acts": acts_t,
            "idx_ab": np.hstack([_wrap_idx(idx[sl, 0].astype(np.int64)),
                                 _wrap_idx(idx[sl, 1].astype(np.int64))]),
            "wplanes": np.ascontiguousarray(wp).astype(F8),
        })
    return in_maps, K_bias


def postprocess(results, K_bias):
    """Sum per-core partials, add bias, subtract global min."""
    total = np.zeros((N_CMP, N_PTS), np.float32)
    for r in results:
        total += r["partial"]
    energies = total + K_bias[:, None]
    out = energies.T - energies.min()
    return np.ascontiguousarray(out.astype(np.float32))


def kernel(node_activations, binarized_learned, edge_endnode_idx,
           _bass_kwargs=None):
    from concourse.bass_utils import run_bass_kernel_spmd

    nc = _get_compiled()
    in_maps, K_bias = prepare_in_maps(
        node_activations, binarized_learned, edge_endnode_idx)
    res = run_bass_kernel_spmd(nc, in_maps, core_ids=list(range(N_CORES)),
                               **(_bass_kwargs or {}))
    out = postprocess(res.results, K_bias)
    kernel.last_results = res
    return out
